# revision 15
# baseline (speedup 1.0000x reference)
"""Trainium2 Bass kernel for nn_DIMESDenseEncoder (GNN message passing).

Self-contained: hardcodes B=16, N=200, U=64, L=3, 8 cores, batch-sharded
(2 graphs per core). Dense edge layout [src*200+dst] with masked diagonal;
feature-major SBUF layout: partition = graph_half*64 + feature.
BatchNorm stats are exact: per-core partial (S, SS) sums fused into the
z-pass via accum_out (S on the psum->sbuf copies, SS on GpSimd
square-accumulate passes), cross-core AllReduce, analytic diagonal
correction. Edge tensors (w, z) live in SBUF as bf16. The edge embed
runs the adjacency through the PE as float32r (full-rate f32).
"""
import os
import numpy as np
import ml_dtypes
import concourse.bass as bass
import concourse.tile as tile
from concourse import bacc, mybir
from concourse.bass_utils import run_bass_kernel_spmd

F32, BF16 = mybir.dt.float32, mybir.dt.bfloat16
F32R = mybir.dt.float32r
AF = mybir.ActivationFunctionType
OP = mybir.AluOpType

B, N, U, L = 16, 200, 64, 3
EPS = 1e-5
NCORES = 8
BC = B // NCORES            # graphs per core
NCOL = N * N                # dense edge cols per graph-half = 40000
CNT_E = B * N * (N - 1)     # global real-edge count
CNT_V = B * N               # global node count

ZCH = 400                   # z-pass chunk cols (2 sources)
NZC = NCOL // ZCH           # z chunks per layer = 100
SSG = 4                     # z chunks per gpsimd sum-of-squares op
PCH_S = 10                  # pooling chunk sources
PCH = PCH_S * N             # pooling chunk cols = 1600
NPOOL = NCOL // PCH         # pooling chunks per layer = 25
PTAIL = 9                   # pool chunks deferred past the e-AllReduce
ACH = 2000                  # apply chunk cols
OCH = 500                   # final-output chunk cols
ECH = 500                   # embed chunk cols

# ---- consts_f32 column layout (host-packed) ----
C_VB0 = 0        # v_lin0_b stacked
C_EW0 = 1        # e_lin0_w stacked
C_EB0 = 2        # e_lin0_b stacked
C_EL1B = 3       # e_lin1_b at all partitions
C_B1 = 4         # v_b1[i] stacked (3 cols)
C_B2 = 7
C_B3 = 10        # v_b3[i]+e_b[i] stacked
C_B4 = 13
C_EG = 16        # e_bn_g[i] p0-63
C_EB = 19
C_VG = 22
C_VB = 25
C_FOLD = 28      # P_fold [128,64]
C_EXP = 92       # E_exp [64,128]
C_I128 = 220     # identity f32 [128,128]
C_S2 = 348       # embed 2-row stationary, rows 0-1 and 2-3 [4,128]
C_VW = 476       # v_wk[i] stacked-two-copies, 12 blocks of 64
C_XW = C_VW + 12 * 64
CF = C_XW + 64

# ---- consts_bf16 columns ----
# 128*i : e_w[i] blockdiag [128,128]; 384:386 e_lin1 blockdiag; 386:514 I128
CB = 514

_CACHE = {}


def _build_consts(inp):
    f = np.zeros((128, CF), np.float32)
    bfc = np.zeros((128, CB), np.float32)

    def stack(v):
        return np.concatenate([v, v]).astype(np.float32)

    f[:, C_VB0] = stack(inp['v_lin0_b'])
    f[:, C_EW0] = stack(inp['e_lin0_w'][0])
    f[:, C_EB0] = stack(inp['e_lin0_b'])
    f[:, C_EL1B] = inp['e_lin1_b'][0]
    for i in range(L):
        f[:, C_B1 + i] = stack(inp['v_b1'][i])
        f[:, C_B2 + i] = stack(inp['v_b2'][i])
        f[:, C_B3 + i] = stack(inp['v_b3'][i] + inp['e_b'][i])
        f[:, C_B4 + i] = stack(inp['v_b4'][i])
        f[:64, C_EG + i] = inp['e_bn_g'][i]
        f[:64, C_EB + i] = inp['e_bn_b'][i]
        f[:64, C_VG + i] = inp['v_bn_g'][i]
        f[:64, C_VB + i] = inp['v_bn_b'][i]
    idx = np.arange(64)
    f[idx, C_FOLD + idx] = 1.0
    f[64 + idx, C_FOLD + idx] = 1.0
    f[idx, C_EXP + idx] = 1.0
    f[idx, C_EXP + 64 + idx] = 1.0
    f[:, C_I128:C_I128 + 128] = np.eye(128, dtype=np.float32)
    # embed stationary: row q -> out 0:64, row q+1 -> out 64:128 (q = 0, 32)
    for q in (0, 32):
        f[q, C_S2:C_S2 + 64] = 1.0
        f[q + 1, C_S2 + 64:C_S2 + 128] = 1.0
    ws = [inp['v_w1'], inp['v_w2'], inp['v_w3'], inp['v_w4']]
    for i in range(L):
        for k in range(4):
            c = C_VW + (i * 4 + k) * 64
            f[:64, c:c + 64] = ws[k][i]
            f[64:, c:c + 64] = ws[k][i]
    f[0:2, C_XW:C_XW + 64] = inp['v_lin0_w']
    f[64:66, C_XW:C_XW + 64] = inp['v_lin0_w']

    for i in range(L):
        bfc[:64, 128 * i:128 * i + 64] = inp['e_w'][i]
        bfc[64:, 128 * i + 64:128 * i + 128] = inp['e_w'][i]
    bfc[:64, 384] = inp['e_lin1_w'][:, 0]
    bfc[64:, 385] = inp['e_lin1_w'][:, 0]
    bfc[:, 386:514] = np.eye(128, dtype=np.float32)
    return f, bfc.astype(ml_dtypes.bfloat16)


def _diag_ap(t_ap, n_src, start=0):
    """AP over diag cols: start, start+201, ... (n_src entries), all 128 parts."""
    return bass.AP(t_ap.tensor, t_ap.offset + start,
                   [[t_ap.ap[0][0], 128], [N + 1, n_src]])


def build_nc():
    nc = bacc.Bacc(None, target_bir_lowering=False, debug=False,
                   num_devices=NCORES)
    x_d = nc.declare_dram_parameter("x", [BC, N, 2], F32, isOutput=False)
    adj_d = nc.declare_dram_parameter("adj", [BC, N, N], F32, isOutput=False)
    cf_d = nc.declare_dram_parameter("cf", [128, CF], F32, isOutput=False)
    cb_d = nc.declare_dram_parameter("cb", [128, CB], BF16, isOutput=False)
    out_d = nc.declare_dram_parameter("out", [BC, N, N], F32, isOutput=True)

    rg = [list(range(NCORES))]

    with tile.TileContext(nc) as tc:
        with (
            tc.tile_pool(name="big", bufs=1) as big,
            tc.tile_pool(name="sb", bufs=1) as sb,
            tc.tile_pool(name="scr", bufs=2) as scr,
            tc.tile_pool(name="ps_z", bufs=4, space="PSUM") as ps_z,
            tc.tile_pool(name="ps_s", bufs=2, space="PSUM") as ps_s,
            tc.tile_pool(name="ps_o", bufs=2, space="PSUM") as ps_o,
            tc.tile_pool(name="dram", bufs=1, space="DRAM") as dram,
        ):
            # ---------- persistent tiles ----------
            w_sb = big.tile([128, NCOL], BF16, tag="w")
            z_sb = big.tile([128, NCOL], BF16, tag="bigz")
            cf = sb.tile([128, CF], F32)
            cb = sb.tile([128, CB], BF16)
            nc.sync.dma_start(cf[:], cf_d[:])
            nc.sync.dma_start(cb[:], cb_d[:])

            h = sb.tile([128, N], F32)
            x1f = sb.tile([128, N], F32)
            x2b = sb.tile([128, N], BF16)
            x3b = sb.tile([128, N], BF16)
            x4b = sb.tile([128, N], BF16)
            x4d = sb.tile([128, 2 * N], BF16)
            pooled = sb.tile([128, N], F32)
            zv = sb.tile([128, N], F32)
            dtile = sb.tile([128, N], F32)
            vjunk = sb.tile([128, N], F32)
            scol = sb.tile([128, NZC], F32)
            sscol = sb.tile([128, NZC // SSG], F32)
            stats_e = sb.tile([128, 6], F32)
            tmp2 = sb.tile([64, 2], F32)
            msq = sb.tile([64, 2], F32)
            var1 = sb.tile([64, 1], F32)
            sd1 = sb.tile([64, 1], F32)
            inv1 = sb.tile([64, 1], F32)
            prm = sb.tile([64, 2], F32)
            scr_e = (msq, var1, sd1, inv1, prm, "psm")
            msqv = sb.tile([64, 2], F32)
            var1v = sb.tile([64, 1], F32)
            sd1v = sb.tile([64, 1], F32)
            inv1v = sb.tile([64, 1], F32)
            prmv = sb.tile([64, 2], F32)
            scr_v = (msqv, var1v, sd1v, inv1v, prmv, "psm")
            pe_sb = sb.tile([128, 2], F32)
            pv_sb = sb.tile([128, 2], F32)
            ar_sb = sb.tile([64, 6], F32)
            zer = sb.tile([2, N], F32)
            nc.vector.memset(zer[:], 0.0)

            def ccol(c, p0=0, p1=128):
                return cf[p0:p1, c:c + 1]

            # ---------- init: h embed ----------
            xt = sb.tile([128, N], F32)
            nc.vector.memset(xt[:], 0.0)
            xr = x_d[:].rearrange("b n c -> b c n")
            nc.sync.dma_start(xt[0:2, :], xr[0])
            nc.sync.dma_start(xt[64:66, :], xr[1])
            ph = ps_s.tile([128, N], F32, tag="psm")
            nc.tensor.matmul(ph[0:64, :], cf[0:2, C_XW:C_XW + 64],
                             xt[0:2, :], start=True, stop=True)
            nc.tensor.matmul(ph[64:128, :], cf[64:66, C_XW:C_XW + 64],
                             xt[64:66, :], start=True, stop=True)
            nc.scalar.activation(h[:], ph[:], AF.Lrelu, bias=ccol(C_VB0),
                                 scale=1.0, alpha=0.01)

            # ---------- init: w embed ----------
            # adj layout [128, 20000]: rows {0,1}=g0/g1 first half,
            # rows {32,33}=g0/g1 second half (quadrant-aligned for the PE).
            # One f32r matmul per chunk broadcasts both graphs.
            adj_sb = big.tile([128, NCOL // 2], F32R, tag="bigz")
            s2t = sb.tile([34, 128], F32R)
            nc.sync.dma_start(s2t[:],
                              cf_d[0:34, C_S2:C_S2 + 128].bitcast(F32R))
            af = adj_d[:].rearrange("b u v -> b (u v)").bitcast(F32R)
            half = NCOL // 2
            qq = half // 4
            dma_engs = [nc.sync, nc.scalar, nc.gpsimd, nc.sync]
            for g in range(2):
                for hh in range(2):
                    p0 = 32 * hh + g
                    for pc in range(4):
                        dma_engs[pc].dma_start(
                            adj_sb[p0:p0 + 1, pc * qq:(pc + 1) * qq],
                            af[g:g + 1, hh * half + pc * qq:
                               hh * half + (pc + 1) * qq])
            for hh in range(2):
                p0 = 32 * hh
                for c in range(half // ECH):
                    pe = ps_z.tile([128, ECH], F32, tag="pz")
                    cs = slice(c * ECH, (c + 1) * ECH)
                    nc.tensor.matmul(pe[:], s2t[p0:p0 + 2, :],
                                     adj_sb[p0:p0 + 2, cs],
                                     start=True, stop=True,
                                     tile_position=(p0, 0))
                    ho = hh * half
                    wcols = w_sb[:, ho + c * ECH:ho + (c + 1) * ECH]
                    if c % 2 == 0:
                        nc.scalar.activation(wcols, pe[:], AF.Lrelu,
                                             bias=ccol(C_EB0),
                                             scale=ccol(C_EW0), alpha=0.01)
                    else:
                        # DVE 3-op lrelu: t = s*pe + b; w = max(t, 0.01*t)
                        et = scr.tile([128, ECH], BF16, tag="sg")
                        eu = scr.tile([128, ECH], BF16, tag="pr")
                        nc.vector.scalar_tensor_tensor(
                            et[:], pe[:], ccol(C_EW0),
                            ccol(C_EB0).broadcast_to([128, ECH]),
                            OP.mult, OP.add)
                        nc.vector.tensor_scalar_mul(eu[:], et[:], 0.01)
                        nc.vector.tensor_tensor(wcols, et[:], eu[:], OP.max)
            nc.vector.memset(_diag_ap(w_sb[:], N), 0.0)

            # ---------- helpers ----------
            def bn_params(ar_ap, gcol, bcol, inv_cnt, out_sb, scratch):
                """ar_ap [64,2]=(S,SS) global -> out_sb [128,2]=(g', b')."""
                msq, var1, sd1, inv1, prm, ptag = scratch
                nc.vector.tensor_scalar_mul(msq[:], ar_ap, inv_cnt)
                nc.vector.tensor_tensor(var1[:], msq[:, 0:1], msq[:, 0:1], OP.mult)
                nc.vector.tensor_tensor(var1[:], msq[:, 1:2], var1[:], OP.subtract)
                nc.vector.tensor_scalar_add(var1[:], var1[:], EPS)
                nc.scalar.sqrt(sd1[:], var1[:])
                nc.vector.reciprocal(inv1[:], sd1[:])
                nc.vector.tensor_tensor(prm[:, 0:1], ccol(gcol, 0, 64), inv1[:],
                                        OP.mult)
                nc.vector.tensor_tensor(prm[:, 1:2], msq[:, 0:1], prm[:, 0:1],
                                        OP.mult)
                nc.vector.tensor_tensor(prm[:, 1:2], ccol(bcol, 0, 64),
                                        prm[:, 1:2], OP.subtract)
                pp = ps_s.tile([128, 2], F32, tag=ptag)
                nc.tensor.matmul(pp[:], cf[0:64, C_EXP:C_EXP + 128], prm[:],
                                 start=True, stop=True)
                nc.scalar.copy(out_sb[:], pp[:])

            # final-output machinery: one apply chunk (ACH cols) = FPG final
            # mm chunks of OCH cols; mm col-base rotates over quadrants
            # {0,32,64,96} so 4 chunks pack one psum tile; one wide ACT adds
            # the bias for all 8 live rows at once -> 2 grouped DMAs.
            of = out_d[:].rearrange("b u v -> b (u v)")
            FPG = ACH // OCH  # final chunks per apply chunk = 4

            def final_out(ac):
                pog = ps_o.tile([128, OCH], F32, tag="pout")
                oc4 = scr.tile([128, OCH], F32, tag="oc4")
                for j in range(FPG):
                    c = ac * FPG + j
                    cols = slice(c * OCH, (c + 1) * OCH)
                    q = 32 * j
                    nc.tensor.matmul(pog[q:q + 2, :], cb[:, 384:386],
                                     w_sb[:, cols], start=True, stop=True,
                                     tile_position=(0, q))
                nc.scalar.activation(oc4[:], pog[:], AF.Identity,
                                     bias=ccol(C_EL1B), scale=1.0)
                # rows {0,32,64,96}+g -> graph g, 4 consecutive OCH blocks
                for g in range(BC):
                    src = bass.AP(oc4.tensor, oc4[:].offset + g * oc4[:].ap[0][0],
                                  [[32 * oc4[:].ap[0][0], 4], [1, OCH]])
                    dst = bass.AP(of.tensor, of.offset + g * NCOL + ac * ACH,
                                  [[OCH, 4], [1, OCH]])
                    nc.sync.dma_start(dst, src)

            # ---------- layers ----------
            stage = int(os.environ.get("KSTAGE", "3"))
            nlay = {0: 0, 1: 1, 2: 1, 3: L}[stage]
            pool_on = stage >= 2
            for i in range(nlay):
                last = (i == L - 1) or not pool_on
                for k, (dst, bcol) in enumerate(((x1f, C_B1 + i), (x2b, C_B2 + i),
                                                 (x3b, C_B3 + i), (x4b, C_B4 + i))):
                    if last and k < 2:
                        continue
                    wc = C_VW + (i * 4 + k) * 64
                    px = ps_s.tile([128, N], F32, tag="psm")
                    nc.tensor.matmul(px[0:64, :], cf[0:64, wc:wc + 64],
                                     h[0:64, :], start=True, stop=True)
                    nc.tensor.matmul(px[64:128, :], cf[64:128, wc:wc + 64],
                                     h[64:128, :], start=True, stop=True,
                                     tile_position=(64, 64))
                    nc.scalar.activation(dst[:], px[:], AF.Identity,
                                         bias=ccol(bcol), scale=1.0)
                nc.vector.tensor_copy(x4d[:, 0:N], x4b[:])
                nc.vector.tensor_copy(x4d[:, N:2 * N], x4b[:])
                x4v = x4b[:].unsqueeze(1).broadcast_to([128, 2, N])

                # z pass interleaved with pooling chunks (keeps PE warm,
                # overlaps ACT sigmoid / DVE prod+max with PE z-matmuls)
                def pool_chunk(c):
                    cols = slice(c * PCH, (c + 1) * PCH)
                    u0 = c * PCH_S
                    sg = scr.tile([128, PCH], BF16, tag="sg")
                    nc.scalar.activation(sg[:], w_sb[:, cols], AF.Sigmoid)
                    pr = scr.tile([128, PCH], BF16, tag="pr")
                    r2 = x2b[:].unsqueeze(1).broadcast_to([128, PCH_S, N])
                    nc.vector.tensor_tensor(
                        pr[:].rearrange("p (u v) -> p u v", v=N),
                        sg[:].rearrange("p (u v) -> p u v", v=N), r2, OP.mult)
                    nc.vector.memset(_diag_ap(pr[:], PCH_S, start=u0), -1e30)
                    pr3 = pr[:].rearrange("p (u v) -> p u v", v=N)
                    ph2 = scr.tile([128, PCH_S * (N // 2)], BF16, tag="oc4")
                    ph23 = ph2[:].rearrange("p (u v) -> p u v", v=N // 2)
                    nc.vector.tensor_tensor(ph23, pr3[:, :, 0:N // 2],
                                            pr3[:, :, N // 2:N], OP.max)
                    nc.vector.reduce_max(
                        pooled[:, u0:u0 + PCH_S], ph23,
                        axis=mybir.AxisListType.X)

                zfrac = NZC // (NCOL // PCH)  # z chunks per pooling chunk
                for c in range(NZC):
                    cols = slice(c * ZCH, (c + 1) * ZCH)
                    u0 = c * 2
                    even = (c % 2 == 0)
                    pz = ps_z.tile([128, ZCH], F32, tag="pz")
                    nc.tensor.matmul(pz[:], cb[:, 128 * i:128 * i + 128],
                                     w_sb[:, cols], start=True, stop=False)
                    pz3 = pz[:].rearrange("p (u v) -> p u v", v=N)
                    r3 = x3b[:, u0:u0 + 2].unsqueeze(2).broadcast_to([128, 2, N])
                    nc.tensor.matmul(pz3, cb[:, 386:514], r3,
                                     start=False, stop=not even)
                    if even:
                        # x4 add via PE; ACT does copy + S-accum
                        nc.tensor.matmul(pz[:], cb[:, 386:514],
                                         x4d[:], start=False, stop=True)
                        nc.scalar.activation(z_sb[:, cols], pz[:], AF.Identity,
                                             bias=0.0, scale=1.0,
                                             accum_out=scol[:, c:c + 1])
                    else:
                        # x4 add fused into the DVE copy, with S-accum
                        nc.vector.scalar_tensor_tensor(
                            z_sb[:, cols].rearrange("p (u v) -> p u v", v=N),
                            pz3, 1.0, x4v, OP.mult, OP.add,
                            accum_out=scol[:, c:c + 1])
                    if c % SSG == SSG - 1:
                        g4 = c // SSG
                        zc = slice(g4 * SSG * ZCH, (c + 1) * ZCH)
                        junk = scr.tile([128, SSG * ZCH], BF16, tag="junk")
                        if g4 % 2 == 0:
                            nc.scalar.activation(junk[:], z_sb[:, zc],
                                                 AF.Square,
                                                 accum_out=sscol[:, g4:g4 + 1])
                        else:
                            nc.vector.scalar_tensor_tensor(
                                junk[:], z_sb[:, zc], 1.0, z_sb[:, zc],
                                OP.mult, OP.mult,
                                accum_out=sscol[:, g4:g4 + 1])
                    # interleave the head pool chunks; the tail runs after
                    # the e-stats AllReduce dispatch to hide its latency
                    if not last and c % zfrac == zfrac - 1:
                        p = c // zfrac
                        if p < NPOOL - PTAIL:
                            pool_chunk(p)
                # e-stats: dense S/SS then diag S_d/SS_d (diag cols of z are
                # exactly x3b+x4b; correction subtracted after AllReduce)
                nc.vector.reduce_sum(stats_e[:, 0:1], scol[:],
                                     axis=mybir.AxisListType.X)
                nc.vector.reduce_sum(stats_e[:, 1:2], sscol[:],
                                     axis=mybir.AxisListType.X)
                nc.vector.scalar_tensor_tensor(dtile[:], x3b[:], 1.0, x4b[:],
                                               OP.mult, OP.add,
                                               accum_out=stats_e[:, 2:3])
                nc.vector.scalar_tensor_tensor(vjunk[:], dtile[:], 1.0, dtile[:],
                                               OP.mult, OP.mult,
                                               accum_out=stats_e[:, 3:4])
                pf_ps = ps_s.tile([64, 6], F32, tag="psm")
                nc.tensor.matmul(pf_ps[:, 0:4], cf[:, C_FOLD:C_FOLD + 64],
                                 stats_e[:, 0:4], start=True, stop=True)
                nc.scalar.copy(ar_sb[:, 0:4], pf_ps[:, 0:4])
                are_in = dram.tile([64, 4], F32, tag=f"arei{i}")
                are_out = dram.tile([64, 4], F32, tag=f"areo{i}")
                nc.sync.dma_start(are_in[:], ar_sb[:, 0:4])
                nc.gpsimd.collective_compute("AllReduce", OP.add,
                                             replica_groups=rg,
                                             ins=[are_in.opt()],
                                             outs=[are_out.opt()])
                ar_e = sb.tile([64, 4], F32, tag="areb")
                nc.gpsimd.dma_start(ar_e[:], are_out[:])

                if not last:
                    # pooling tail + v-stats + v AllReduce, all overlapping
                    # the e AllReduce latency
                    for p in range(NPOOL - PTAIL, NPOOL):
                        pool_chunk(p)
                    nc.vector.scalar_tensor_tensor(zv[:], x1f[:], 1.0, pooled[:],
                                                   OP.mult, OP.add,
                                                   accum_out=stats_e[:, 4:5])
                    nc.vector.scalar_tensor_tensor(vjunk[:], zv[:], 1.0, zv[:],
                                                   OP.mult, OP.mult,
                                                   accum_out=stats_e[:, 5:6])
                    pv_ps = ps_s.tile([64, 6], F32, tag="psm")
                    nc.tensor.matmul(pv_ps[:, 0:2], cf[:, C_FOLD:C_FOLD + 64],
                                     stats_e[:, 4:6], start=True, stop=True)
                    nc.scalar.copy(ar_sb[:, 4:6], pv_ps[:, 0:2])
                    arv_in = dram.tile([64, 2], F32, tag=f"arvi{i}")
                    arv_out = dram.tile([64, 2], F32, tag=f"arvo{i}")
                    nc.sync.dma_start(arv_in[:], ar_sb[:, 4:6])
                    nc.gpsimd.collective_compute("AllReduce", OP.add,
                                                 replica_groups=rg,
                                                 ins=[arv_in.opt()],
                                                 outs=[arv_out.opt()])
                    ar_v = sb.tile([64, 2], F32, tag="arvb")
                    nc.gpsimd.dma_start(ar_v[:], arv_out[:])

                # e-BN params; apply to w
                nc.vector.tensor_tensor(tmp2[:], ar_e[:, 0:2], ar_e[:, 2:4],
                                        OP.subtract)
                bn_params(tmp2[:], C_EG + i, C_EB + i, 1.0 / CNT_E, pe_sb, scr_e)
                lr0 = None
                for c in range(NCOL // ACH):
                    cols = slice(c * ACH, (c + 1) * ACH)
                    lr = scr.tile([128, ACH], BF16, tag="lr")
                    nc.scalar.activation(lr[:], z_sb[:, cols], AF.Lrelu,
                                         bias=pe_sb[:, 1:2], scale=pe_sb[:, 0:1],
                                         alpha=0.01)
                    if lr0 is None:
                        lr0 = lr
                    nc.vector.tensor_tensor(w_sb[:, cols], w_sb[:, cols],
                                            lr[:], OP.add)
                    if i == L - 1:
                        final_out(c)
                if not last:
                    nc.vector.memset(_diag_ap(w_sb[:], N), 0.0)
                    # WAW hook: pins the v-params chain behind the first apply
                    # chunk so its ACT ops can't be scheduled ahead of the
                    # apply (the sim underestimates AllReduce latency)
                    nc.vector.tensor_copy(msqv[0:1, 0:1], lr0[0:1, 0:1])
                    bn_params(ar_v[:], C_VG + i, C_VB + i, 1.0 / CNT_V, pv_sb, scr_v)
                    hup = sb.tile([128, N], F32, tag="hup")
                    nc.scalar.activation(hup[:], zv[:], AF.Lrelu,
                                         bias=pv_sb[:, 1:2], scale=pv_sb[:, 0:1],
                                         alpha=0.01)
                    nc.vector.tensor_tensor(h[:], h[:], hup[:], OP.add)

            # final-output epilogue: covered by final_out() calls inside
            # the last apply loop; here only the diagonal zeroing remains.
            dd = bass.AP(of.tensor, of.offset, [[NCOL, BC], [N + 1, N]])
            nc.sync.dma_start(dd, zer[:])
    nc.compile()
    return nc


def _get_nc():
    if "nc" not in _CACHE:
        _CACHE["nc"] = build_nc()
    return _CACHE["nc"]


def run(inputs, **kw):
    inp = {k: np.asarray(v, np.float32) for k, v in inputs.items()}
    cfh, cbh = _build_consts(inp)
    nc = _get_nc()
    in_maps = []
    for c in range(NCORES):
        sl = slice(c * BC, (c + 1) * BC)
        in_maps.append({
            "x": np.ascontiguousarray(inp['x'][sl]),
            "adj": np.ascontiguousarray(inp['adj'][sl]),
            "cf": cfh, "cb": cbh,
        })
    res = run_bass_kernel_spmd(nc, in_maps, core_ids=list(range(NCORES)), **kw)
    out = np.concatenate([res.results[c]["out"] for c in range(NCORES)], axis=0)
    return out, res


def kernel(**inputs) -> np.ndarray:
    out, _ = run(inputs)
    return out


# revision 16
# speedup vs baseline: 1.0673x; 1.0673x over previous
"""Trainium2 Bass kernel for nn_DIMESDenseEncoder (GNN message passing).

Self-contained: hardcodes B=16, N=200, U=64, L=3, 8 cores, batch-sharded
(2 graphs per core). Dense edge layout [src*200+dst] with masked diagonal;
feature-major SBUF layout: partition = graph_half*64 + feature.
BatchNorm stats are exact: per-core partial (S, SS) sums fused into the
z-pass via accum_out (S on the psum->sbuf copies, SS on GpSimd
square-accumulate passes), cross-core AllReduce, analytic diagonal
correction. Edge tensors (w, z) live in SBUF as bf16. The edge embed
runs the adjacency through the PE as float32r (full-rate f32).
"""
import os
import numpy as np
import ml_dtypes
import concourse.bass as bass
import concourse.tile as tile
from concourse import bacc, mybir
from concourse.bass_utils import run_bass_kernel_spmd

F32, BF16 = mybir.dt.float32, mybir.dt.bfloat16
F32R = mybir.dt.float32r
AF = mybir.ActivationFunctionType
OP = mybir.AluOpType

B, N, U, L = 16, 200, 64, 3
EPS = 1e-5
NCORES = 8
BC = B // NCORES            # graphs per core
NCOL = N * N                # dense edge cols per graph-half = 40000
CNT_E = B * N * (N - 1)     # global real-edge count
CNT_V = B * N               # global node count

ZCH = 400                   # z-pass chunk cols (2 sources)
NZC = NCOL // ZCH           # z chunks per layer = 100
SSG = 4                     # z chunks per gpsimd sum-of-squares op
PCH_S = 10                  # pooling chunk sources
PCH = PCH_S * N             # pooling chunk cols = 1600
NPOOL = NCOL // PCH         # pooling chunks per layer = 25
PTAIL = 12                  # pool chunks deferred past the e-AllReduce
ACH = 2000                  # apply chunk cols
OCH = 500                   # final-output chunk cols
ECH = 500                   # embed chunk cols

# ---- consts_f32 column layout (host-packed) ----
C_VB0 = 0        # v_lin0_b stacked
C_EW0 = 1        # e_lin0_w stacked
C_EB0 = 2        # e_lin0_b stacked
C_EL1B = 3       # e_lin1_b at all partitions
C_B1 = 4         # v_b1[i] stacked (3 cols)
C_B2 = 7
C_B3 = 10        # v_b3[i]+e_b[i] stacked
C_B4 = 13
C_EG = 16        # e_bn_g[i] p0-63
C_EB = 19
C_VG = 22
C_VB = 25
C_FOLD = 28      # P_fold [128,64]
C_EXP = 92       # E_exp [64,128]
C_I128 = 220     # identity f32 [128,128]
C_S2 = 348       # embed 2-row stationary, rows 0-1 and 2-3 [4,128]
C_VW = 476       # v_wk[i] stacked-two-copies, 12 blocks of 64
C_XW = C_VW + 12 * 64
CF = C_XW + 64

# ---- consts_bf16 columns ----
# 128*i : e_w[i] blockdiag [128,128]; 384:386 e_lin1 blockdiag; 386:514 I128
CB = 514

_CACHE = {}


def _build_consts(inp):
    f = np.zeros((128, CF), np.float32)
    bfc = np.zeros((128, CB), np.float32)

    def stack(v):
        return np.concatenate([v, v]).astype(np.float32)

    f[:, C_VB0] = stack(inp['v_lin0_b'])
    f[:, C_EW0] = stack(inp['e_lin0_w'][0])
    f[:, C_EB0] = stack(inp['e_lin0_b'])
    f[:, C_EL1B] = inp['e_lin1_b'][0]
    for i in range(L):
        f[:, C_B1 + i] = stack(inp['v_b1'][i])
        f[:, C_B2 + i] = stack(inp['v_b2'][i])
        f[:, C_B3 + i] = stack(inp['v_b3'][i] + inp['e_b'][i])
        f[:, C_B4 + i] = stack(inp['v_b4'][i])
        f[:64, C_EG + i] = inp['e_bn_g'][i]
        f[:64, C_EB + i] = inp['e_bn_b'][i]
        f[:64, C_VG + i] = inp['v_bn_g'][i]
        f[:64, C_VB + i] = inp['v_bn_b'][i]
    idx = np.arange(64)
    f[idx, C_FOLD + idx] = 1.0
    f[64 + idx, C_FOLD + idx] = 1.0
    f[idx, C_EXP + idx] = 1.0
    f[idx, C_EXP + 64 + idx] = 1.0
    f[:, C_I128:C_I128 + 128] = np.eye(128, dtype=np.float32)
    # embed stationary: row q -> out 0:64, row q+1 -> out 64:128 (q = 0, 32)
    for q in (0, 32):
        f[q, C_S2:C_S2 + 64] = 1.0
        f[q + 1, C_S2 + 64:C_S2 + 128] = 1.0
    ws = [inp['v_w1'], inp['v_w2'], inp['v_w3'], inp['v_w4']]
    for i in range(L):
        for k in range(4):
            c = C_VW + (i * 4 + k) * 64
            f[:64, c:c + 64] = ws[k][i]
            f[64:, c:c + 64] = ws[k][i]
    f[0:2, C_XW:C_XW + 64] = inp['v_lin0_w']
    f[64:66, C_XW:C_XW + 64] = inp['v_lin0_w']

    for i in range(L):
        bfc[:64, 128 * i:128 * i + 64] = inp['e_w'][i]
        bfc[64:, 128 * i + 64:128 * i + 128] = inp['e_w'][i]
    bfc[:64, 384] = inp['e_lin1_w'][:, 0]
    bfc[64:, 385] = inp['e_lin1_w'][:, 0]
    bfc[:, 386:514] = np.eye(128, dtype=np.float32)
    return f, bfc.astype(ml_dtypes.bfloat16)


def _diag_ap(t_ap, n_src, start=0):
    """AP over diag cols: start, start+201, ... (n_src entries), all 128 parts."""
    return bass.AP(t_ap.tensor, t_ap.offset + start,
                   [[t_ap.ap[0][0], 128], [N + 1, n_src]])


def build_nc():
    nc = bacc.Bacc(None, target_bir_lowering=False, debug=False,
                   num_devices=NCORES)
    x_d = nc.declare_dram_parameter("x", [BC, N, 2], F32, isOutput=False)
    adj_d = nc.declare_dram_parameter("adj", [BC, N, N], F32, isOutput=False)
    cf_d = nc.declare_dram_parameter("cf", [128, CF], F32, isOutput=False)
    cb_d = nc.declare_dram_parameter("cb", [128, CB], BF16, isOutput=False)
    out_d = nc.declare_dram_parameter("out", [BC, N, N], F32, isOutput=True)

    rg = [list(range(NCORES))]

    with tile.TileContext(nc) as tc:
        with (
            tc.tile_pool(name="big", bufs=1) as big,
            tc.tile_pool(name="sb", bufs=1) as sb,
            tc.tile_pool(name="scr", bufs=2) as scr,
            tc.tile_pool(name="ps_z", bufs=4, space="PSUM") as ps_z,
            tc.tile_pool(name="ps_s", bufs=2, space="PSUM") as ps_s,
            tc.tile_pool(name="ps_o", bufs=2, space="PSUM") as ps_o,
            tc.tile_pool(name="dram", bufs=1, space="DRAM") as dram,
        ):
            # ---------- persistent tiles ----------
            w_sb = big.tile([128, NCOL], BF16, tag="w")
            z_sb = big.tile([128, NCOL], BF16, tag="bigz")
            cf = sb.tile([128, CF], F32)
            cb = sb.tile([128, CB], BF16)
            nc.sync.dma_start(cf[:], cf_d[:])
            nc.sync.dma_start(cb[:], cb_d[:])

            h = sb.tile([128, N], F32)
            x1f = sb.tile([128, N], F32)
            x2b = sb.tile([128, N], BF16)
            x3b = sb.tile([128, N], BF16)
            x4b = sb.tile([128, N], BF16)
            x4d = sb.tile([128, 2 * N], BF16)
            pooled = sb.tile([128, N], F32)
            zv = sb.tile([128, N], F32)
            dtile = sb.tile([128, N], F32)
            vjunk = sb.tile([128, N], F32)
            scol = sb.tile([128, NZC], F32)
            sscol = sb.tile([128, NZC // SSG], F32)
            stats_e = sb.tile([128, 6], F32)
            tmp2 = sb.tile([64, 2], F32)
            msq = sb.tile([64, 2], F32)
            var1 = sb.tile([64, 1], F32)
            sd1 = sb.tile([64, 1], F32)
            inv1 = sb.tile([64, 1], F32)
            prm = sb.tile([64, 2], F32)
            scr_e = (msq, var1, sd1, inv1, prm, "psm")
            msqv = sb.tile([64, 2], F32)
            var1v = sb.tile([64, 1], F32)
            sd1v = sb.tile([64, 1], F32)
            inv1v = sb.tile([64, 1], F32)
            prmv = sb.tile([64, 2], F32)
            scr_v = (msqv, var1v, sd1v, inv1v, prmv, "psm")
            pe_sb = sb.tile([128, 2], F32)
            pv_sb = sb.tile([128, 2], F32)
            ar_sb = sb.tile([64, 6], F32)
            zer = sb.tile([2, N], F32)
            nc.vector.memset(zer[:], 0.0)

            def ccol(c, p0=0, p1=128):
                return cf[p0:p1, c:c + 1]

            # ---------- init: h embed ----------
            xt = sb.tile([128, N], F32)
            nc.vector.memset(xt[:], 0.0)
            xr = x_d[:].rearrange("b n c -> b c n")
            nc.sync.dma_start(xt[0:2, :], xr[0])
            nc.sync.dma_start(xt[64:66, :], xr[1])
            ph = ps_s.tile([128, N], F32, tag="psm")
            nc.tensor.matmul(ph[0:64, :], cf[0:2, C_XW:C_XW + 64],
                             xt[0:2, :], start=True, stop=True)
            nc.tensor.matmul(ph[64:128, :], cf[64:66, C_XW:C_XW + 64],
                             xt[64:66, :], start=True, stop=True)
            nc.scalar.activation(h[:], ph[:], AF.Lrelu, bias=ccol(C_VB0),
                                 scale=1.0, alpha=0.01)

            # ---------- init: w embed ----------
            # adj layout [128, 20000]: rows {0,1}=g0/g1 first half,
            # rows {32,33}=g0/g1 second half (quadrant-aligned for the PE).
            # One f32r matmul per chunk broadcasts both graphs.
            adj_sb = big.tile([128, NCOL // 2], F32R, tag="bigz")
            s2t = sb.tile([34, 128], F32R)
            nc.sync.dma_start(s2t[:],
                              cf_d[0:34, C_S2:C_S2 + 128].bitcast(F32R))
            af = adj_d[:].rearrange("b u v -> b (u v)").bitcast(F32R)
            half = NCOL // 2
            qq = half // 4
            dma_engs = [nc.sync, nc.scalar, nc.gpsimd, nc.sync]
            for g in range(2):
                for hh in range(2):
                    p0 = 32 * hh + g
                    for pc in range(4):
                        dma_engs[pc].dma_start(
                            adj_sb[p0:p0 + 1, pc * qq:(pc + 1) * qq],
                            af[g:g + 1, hh * half + pc * qq:
                               hh * half + (pc + 1) * qq])
            for hh in range(2):
                p0 = 32 * hh
                for c in range(half // ECH):
                    pe = ps_z.tile([128, ECH], F32, tag="pz")
                    cs = slice(c * ECH, (c + 1) * ECH)
                    nc.tensor.matmul(pe[:], s2t[p0:p0 + 2, :],
                                     adj_sb[p0:p0 + 2, cs],
                                     start=True, stop=True,
                                     tile_position=(p0, 0))
                    ho = hh * half
                    wcols = w_sb[:, ho + c * ECH:ho + (c + 1) * ECH]
                    if c % 2 == 0:
                        nc.scalar.activation(wcols, pe[:], AF.Lrelu,
                                             bias=ccol(C_EB0),
                                             scale=ccol(C_EW0), alpha=0.01)
                    else:
                        # DVE 3-op lrelu: t = s*pe + b; w = max(t, 0.01*t)
                        et = scr.tile([128, ECH], BF16, tag="sg")
                        eu = scr.tile([128, ECH], BF16, tag="pr")
                        nc.vector.scalar_tensor_tensor(
                            et[:], pe[:], ccol(C_EW0),
                            ccol(C_EB0).broadcast_to([128, ECH]),
                            OP.mult, OP.add)
                        nc.vector.tensor_scalar_mul(eu[:], et[:], 0.01)
                        nc.vector.tensor_tensor(wcols, et[:], eu[:], OP.max)
            nc.vector.memset(_diag_ap(w_sb[:], N), 0.0)

            # ---------- helpers ----------
            def bn_params(ar_ap, gcol, bcol, inv_cnt, out_sb, scratch):
                """ar_ap [64,2]=(S,SS) global -> out_sb [128,2]=(g', b')."""
                msq, var1, sd1, inv1, prm, ptag = scratch
                nc.vector.tensor_scalar_mul(msq[:], ar_ap, inv_cnt)
                nc.vector.tensor_tensor(var1[:], msq[:, 0:1], msq[:, 0:1], OP.mult)
                nc.vector.tensor_tensor(var1[:], msq[:, 1:2], var1[:], OP.subtract)
                nc.vector.tensor_scalar_add(var1[:], var1[:], EPS)
                nc.scalar.sqrt(sd1[:], var1[:])
                nc.vector.reciprocal(inv1[:], sd1[:])
                nc.vector.tensor_tensor(prm[:, 0:1], ccol(gcol, 0, 64), inv1[:],
                                        OP.mult)
                nc.vector.tensor_tensor(prm[:, 1:2], msq[:, 0:1], prm[:, 0:1],
                                        OP.mult)
                nc.vector.tensor_tensor(prm[:, 1:2], ccol(bcol, 0, 64),
                                        prm[:, 1:2], OP.subtract)
                pp = ps_s.tile([128, 2], F32, tag=ptag)
                nc.tensor.matmul(pp[:], cf[0:64, C_EXP:C_EXP + 128], prm[:],
                                 start=True, stop=True)
                nc.scalar.copy(out_sb[:], pp[:])

            # final-output machinery: one apply chunk (ACH cols) = FPG final
            # mm chunks of OCH cols; mm col-base rotates over quadrants
            # {0,32,64,96} so 4 chunks pack one psum tile; one wide ACT adds
            # the bias for all 8 live rows at once -> 2 grouped DMAs.
            of = out_d[:].rearrange("b u v -> b (u v)")
            FPG = ACH // OCH  # final chunks per apply chunk = 4

            def final_out(ac):
                pog = ps_o.tile([128, OCH], F32, tag="pout")
                oc4 = scr.tile([128, OCH], F32, tag="oc4")
                for j in range(FPG):
                    c = ac * FPG + j
                    cols = slice(c * OCH, (c + 1) * OCH)
                    q = 32 * j
                    nc.tensor.matmul(pog[q:q + 2, :], cb[:, 384:386],
                                     w_sb[:, cols], start=True, stop=True,
                                     tile_position=(0, q))
                nc.scalar.activation(oc4[:], pog[:], AF.Identity,
                                     bias=ccol(C_EL1B), scale=1.0)
                # rows {0,32,64,96}+g -> graph g, 4 consecutive OCH blocks
                for g in range(BC):
                    src = bass.AP(oc4.tensor, oc4[:].offset + g * oc4[:].ap[0][0],
                                  [[32 * oc4[:].ap[0][0], 4], [1, OCH]])
                    dst = bass.AP(of.tensor, of.offset + g * NCOL + ac * ACH,
                                  [[OCH, 4], [1, OCH]])
                    nc.sync.dma_start(dst, src)

            # ---------- layers ----------
            stage = int(os.environ.get("KSTAGE", "3"))
            nlay = {0: 0, 1: 1, 2: 1, 3: L}[stage]
            pool_on = stage >= 2
            for i in range(nlay):
                last = (i == L - 1) or not pool_on
                for k, (dst, bcol) in enumerate(((x1f, C_B1 + i), (x2b, C_B2 + i),
                                                 (x3b, C_B3 + i), (x4b, C_B4 + i))):
                    if last and k < 2:
                        continue
                    wc = C_VW + (i * 4 + k) * 64
                    px = ps_s.tile([128, N], F32, tag="psm")
                    nc.tensor.matmul(px[0:64, :], cf[0:64, wc:wc + 64],
                                     h[0:64, :], start=True, stop=True)
                    nc.tensor.matmul(px[64:128, :], cf[64:128, wc:wc + 64],
                                     h[64:128, :], start=True, stop=True,
                                     tile_position=(64, 64))
                    nc.scalar.activation(dst[:], px[:], AF.Identity,
                                         bias=ccol(bcol), scale=1.0)
                nc.vector.tensor_copy(x4d[:, 0:N], x4b[:])
                nc.vector.tensor_copy(x4d[:, N:2 * N], x4b[:])
                x4v = x4b[:].unsqueeze(1).broadcast_to([128, 2, N])

                # z pass interleaved with pooling chunks (keeps PE warm,
                # overlaps ACT sigmoid / DVE prod+max with PE z-matmuls)
                def pool_chunk(c):
                    cols = slice(c * PCH, (c + 1) * PCH)
                    u0 = c * PCH_S
                    sg = scr.tile([128, PCH], BF16, tag="sg")
                    nc.scalar.activation(sg[:], w_sb[:, cols], AF.Sigmoid)
                    pr = scr.tile([128, PCH], BF16, tag="pr")
                    r2 = x2b[:].unsqueeze(1).broadcast_to([128, PCH_S, N])
                    nc.vector.tensor_tensor(
                        pr[:].rearrange("p (u v) -> p u v", v=N),
                        sg[:].rearrange("p (u v) -> p u v", v=N), r2, OP.mult)
                    nc.vector.memset(_diag_ap(pr[:], PCH_S, start=u0), -1e30)
                    pr3 = pr[:].rearrange("p (u v) -> p u v", v=N)
                    ph2 = scr.tile([128, PCH_S * (N // 2)], BF16, tag="oc4")
                    ph23 = ph2[:].rearrange("p (u v) -> p u v", v=N // 2)
                    nc.vector.tensor_tensor(ph23, pr3[:, :, 0:N // 2],
                                            pr3[:, :, N // 2:N], OP.max)
                    nc.vector.reduce_max(
                        pooled[:, u0:u0 + PCH_S], ph23,
                        axis=mybir.AxisListType.X)

                zfrac = NZC // (NCOL // PCH)  # z chunks per pooling chunk
                for c in range(NZC):
                    cols = slice(c * ZCH, (c + 1) * ZCH)
                    u0 = c * 2
                    even = (c % 2 == 0)
                    pz = ps_z.tile([128, ZCH], F32, tag="pz")
                    nc.tensor.matmul(pz[:], cb[:, 128 * i:128 * i + 128],
                                     w_sb[:, cols], start=True, stop=False)
                    pz3 = pz[:].rearrange("p (u v) -> p u v", v=N)
                    r3 = x3b[:, u0:u0 + 2].unsqueeze(2).broadcast_to([128, 2, N])
                    nc.tensor.matmul(pz3, cb[:, 386:514], r3,
                                     start=False, stop=not even)
                    if even:
                        # x4 add via PE; ACT does copy + S-accum
                        nc.tensor.matmul(pz[:], cb[:, 386:514],
                                         x4d[:], start=False, stop=True)
                        nc.scalar.activation(z_sb[:, cols], pz[:], AF.Identity,
                                             bias=0.0, scale=1.0,
                                             accum_out=scol[:, c:c + 1])
                    else:
                        # x4 add fused into the DVE copy, with S-accum
                        nc.vector.scalar_tensor_tensor(
                            z_sb[:, cols].rearrange("p (u v) -> p u v", v=N),
                            pz3, 1.0, x4v, OP.mult, OP.add,
                            accum_out=scol[:, c:c + 1])
                    if c % SSG == SSG - 1:
                        g4 = c // SSG
                        zc = slice(g4 * SSG * ZCH, (c + 1) * ZCH)
                        junk = scr.tile([128, SSG * ZCH], BF16, tag="lr")
                        if g4 % 3 != 2:
                            nc.scalar.activation(junk[:], z_sb[:, zc],
                                                 AF.Square,
                                                 accum_out=sscol[:, g4:g4 + 1])
                        else:
                            nc.vector.scalar_tensor_tensor(
                                junk[:], z_sb[:, zc], 1.0, z_sb[:, zc],
                                OP.mult, OP.mult,
                                accum_out=sscol[:, g4:g4 + 1])
                    # interleave the head pool chunks; the tail runs after
                    # the e-stats AllReduce dispatch to hide its latency
                    if not last and c % zfrac == zfrac - 1:
                        p = c // zfrac
                        if p < NPOOL - PTAIL:
                            pool_chunk(p)
                # e-stats: dense S/SS then diag S_d/SS_d (diag cols of z are
                # exactly x3b+x4b; correction subtracted after AllReduce)
                nc.vector.reduce_sum(stats_e[:, 0:1], scol[:],
                                     axis=mybir.AxisListType.X)
                nc.vector.reduce_sum(stats_e[:, 1:2], sscol[:],
                                     axis=mybir.AxisListType.X)
                nc.vector.scalar_tensor_tensor(dtile[:], x3b[:], 1.0, x4b[:],
                                               OP.mult, OP.add,
                                               accum_out=stats_e[:, 2:3])
                nc.vector.scalar_tensor_tensor(vjunk[:], dtile[:], 1.0, dtile[:],
                                               OP.mult, OP.mult,
                                               accum_out=stats_e[:, 3:4])
                pf_ps = ps_s.tile([64, 6], F32, tag="psm")
                nc.tensor.matmul(pf_ps[:, 0:4], cf[:, C_FOLD:C_FOLD + 64],
                                 stats_e[:, 0:4], start=True, stop=True)
                nc.scalar.copy(ar_sb[:, 0:4], pf_ps[:, 0:4])
                are_in = dram.tile([64, 4], F32, tag=f"arei{i}")
                are_out = dram.tile([64, 4], F32, tag=f"areo{i}")
                nc.sync.dma_start(are_in[:], ar_sb[:, 0:4])
                nc.gpsimd.collective_compute("AllReduce", OP.add,
                                             replica_groups=rg,
                                             ins=[are_in.opt()],
                                             outs=[are_out.opt()])
                ar_e = sb.tile([64, 4], F32, tag="areb")
                nc.gpsimd.dma_start(ar_e[:], are_out[:])

                if not last:
                    # pooling tail + v-stats + v AllReduce, all overlapping
                    # the e AllReduce latency
                    for p in range(NPOOL - PTAIL, NPOOL):
                        pool_chunk(p)
                    nc.vector.scalar_tensor_tensor(zv[:], x1f[:], 1.0, pooled[:],
                                                   OP.mult, OP.add,
                                                   accum_out=stats_e[:, 4:5])
                    nc.vector.scalar_tensor_tensor(vjunk[:], zv[:], 1.0, zv[:],
                                                   OP.mult, OP.mult,
                                                   accum_out=stats_e[:, 5:6])
                    pv_ps = ps_s.tile([64, 6], F32, tag="psm")
                    nc.tensor.matmul(pv_ps[:, 0:2], cf[:, C_FOLD:C_FOLD + 64],
                                     stats_e[:, 4:6], start=True, stop=True)
                    nc.scalar.copy(ar_sb[:, 4:6], pv_ps[:, 0:2])
                    arv_in = dram.tile([64, 2], F32, tag=f"arvi{i}")
                    arv_out = dram.tile([64, 2], F32, tag=f"arvo{i}")
                    nc.sync.dma_start(arv_in[:], ar_sb[:, 4:6])
                    nc.gpsimd.collective_compute("AllReduce", OP.add,
                                                 replica_groups=rg,
                                                 ins=[arv_in.opt()],
                                                 outs=[arv_out.opt()])
                    ar_v = sb.tile([64, 2], F32, tag="arvb")
                    nc.gpsimd.dma_start(ar_v[:], arv_out[:])

                # e-BN params; apply to w
                nc.vector.tensor_tensor(tmp2[:], ar_e[:, 0:2], ar_e[:, 2:4],
                                        OP.subtract)
                bn_params(tmp2[:], C_EG + i, C_EB + i, 1.0 / CNT_E, pe_sb, scr_e)
                lr0 = None
                for c in range(NCOL // ACH):
                    cols = slice(c * ACH, (c + 1) * ACH)
                    lr = scr.tile([128, ACH], BF16, tag="lr")
                    nc.scalar.activation(lr[:], z_sb[:, cols], AF.Lrelu,
                                         bias=pe_sb[:, 1:2], scale=pe_sb[:, 0:1],
                                         alpha=0.01)
                    if lr0 is None:
                        lr0 = lr
                    nc.vector.tensor_tensor(w_sb[:, cols], w_sb[:, cols],
                                            lr[:], OP.add)
                    if i == L - 1:
                        final_out(c)
                if not last:
                    nc.vector.memset(_diag_ap(w_sb[:], N), 0.0)
                    # WAW hook: pins the v-params chain behind the first apply
                    # chunk so its ACT ops can't be scheduled ahead of the
                    # apply (the sim underestimates AllReduce latency)
                    nc.vector.tensor_copy(msqv[0:1, 0:1], lr0[0:1, 0:1])
                    bn_params(ar_v[:], C_VG + i, C_VB + i, 1.0 / CNT_V, pv_sb, scr_v)
                    hup = sb.tile([128, N], F32, tag="hup")
                    nc.scalar.activation(hup[:], zv[:], AF.Lrelu,
                                         bias=pv_sb[:, 1:2], scale=pv_sb[:, 0:1],
                                         alpha=0.01)
                    nc.vector.tensor_tensor(h[:], h[:], hup[:], OP.add)

            # final-output epilogue: covered by final_out() calls inside
            # the last apply loop; here only the diagonal zeroing remains.
            dd = bass.AP(of.tensor, of.offset, [[NCOL, BC], [N + 1, N]])
            nc.sync.dma_start(dd, zer[:])
    nc.compile()
    return nc


def _get_nc():
    if "nc" not in _CACHE:
        _CACHE["nc"] = build_nc()
    return _CACHE["nc"]


def run(inputs, **kw):
    inp = {k: np.asarray(v, np.float32) for k, v in inputs.items()}
    cfh, cbh = _build_consts(inp)
    nc = _get_nc()
    in_maps = []
    for c in range(NCORES):
        sl = slice(c * BC, (c + 1) * BC)
        in_maps.append({
            "x": np.ascontiguousarray(inp['x'][sl]),
            "adj": np.ascontiguousarray(inp['adj'][sl]),
            "cf": cfh, "cb": cbh,
        })
    res = run_bass_kernel_spmd(nc, in_maps, core_ids=list(range(NCORES)), **kw)
    out = np.concatenate([res.results[c]["out"] for c in range(NCORES)], axis=0)
    return out, res


def kernel(**inputs) -> np.ndarray:
    out, _ = run(inputs)
    return out


# revision 17
# speedup vs baseline: 1.0720x; 1.0044x over previous
"""Trainium2 Bass kernel for nn_DIMESDenseEncoder (GNN message passing).

Self-contained: hardcodes B=16, N=200, U=64, L=3, 8 cores, batch-sharded
(2 graphs per core). Dense edge layout [src*200+dst] with masked diagonal;
feature-major SBUF layout: partition = graph_half*64 + feature.
BatchNorm stats are exact: per-core partial (S, SS) sums fused into the
z-pass via accum_out (S on the psum->sbuf copies, SS on GpSimd
square-accumulate passes), cross-core AllReduce, analytic diagonal
correction. Edge tensors (w, z) live in SBUF as bf16. The edge embed
runs the adjacency through the PE as float32r (full-rate f32).
"""
import os
import numpy as np
import ml_dtypes
import concourse.bass as bass
import concourse.tile as tile
from concourse import bacc, mybir
from concourse.bass_utils import run_bass_kernel_spmd

F32, BF16 = mybir.dt.float32, mybir.dt.bfloat16
F32R = mybir.dt.float32r
AF = mybir.ActivationFunctionType
OP = mybir.AluOpType

B, N, U, L = 16, 200, 64, 3
EPS = 1e-5
NCORES = 8
BC = B // NCORES            # graphs per core
NCOL = N * N                # dense edge cols per graph-half = 40000
CNT_E = B * N * (N - 1)     # global real-edge count
CNT_V = B * N               # global node count

ZCH = 400                   # z-pass chunk cols (2 sources)
NZC = NCOL // ZCH           # z chunks per layer = 100
SSG = 4                     # z chunks per gpsimd sum-of-squares op
PCH_S = 10                  # pooling chunk sources
PCH = PCH_S * N             # pooling chunk cols = 1600
NPOOL = NCOL // PCH         # pooling chunks per layer = 25
PTAIL = 12                  # pool chunks deferred past the e-AllReduce
ACH = 2000                  # apply chunk cols
OCH = 500                   # final-output chunk cols
ECH = 500                   # embed chunk cols

# ---- consts_f32 column layout (host-packed) ----
C_VB0 = 0        # v_lin0_b stacked
C_EW0 = 1        # e_lin0_w stacked
C_EB0 = 2        # e_lin0_b stacked
C_EL1B = 3       # e_lin1_b at all partitions
C_B1 = 4         # v_b1[i] stacked (3 cols)
C_B2 = 7
C_B3 = 10        # v_b3[i]+e_b[i] stacked
C_B4 = 13
C_EG = 16        # e_bn_g[i] p0-63
C_EB = 19
C_VG = 22
C_VB = 25
C_FOLD = 28      # P_fold [128,64]
C_EXP = 92       # E_exp [64,128]
C_I128 = 220     # identity f32 [128,128]
C_S2 = 348       # embed 2-row stationary, rows 0-1 and 2-3 [4,128]
C_VW = 476       # v_wk[i] stacked-two-copies, 12 blocks of 64
C_XW = C_VW + 12 * 64
CF = C_XW + 64

# ---- consts_bf16 columns ----
# 128*i : e_w[i] blockdiag [128,128]; 384:386 e_lin1 blockdiag; 386:514 I128
CB = 514

_CACHE = {}


def _build_consts(inp):
    f = np.zeros((128, CF), np.float32)
    bfc = np.zeros((128, CB), np.float32)

    def stack(v):
        return np.concatenate([v, v]).astype(np.float32)

    f[:, C_VB0] = stack(inp['v_lin0_b'])
    f[:, C_EW0] = stack(inp['e_lin0_w'][0])
    f[:, C_EB0] = stack(inp['e_lin0_b'])
    f[:, C_EL1B] = inp['e_lin1_b'][0]
    for i in range(L):
        f[:, C_B1 + i] = stack(inp['v_b1'][i])
        f[:, C_B2 + i] = stack(inp['v_b2'][i])
        f[:, C_B3 + i] = stack(inp['v_b3'][i] + inp['e_b'][i])
        f[:, C_B4 + i] = stack(inp['v_b4'][i])
        f[:64, C_EG + i] = inp['e_bn_g'][i]
        f[:64, C_EB + i] = inp['e_bn_b'][i]
        f[:64, C_VG + i] = inp['v_bn_g'][i]
        f[:64, C_VB + i] = inp['v_bn_b'][i]
    idx = np.arange(64)
    f[idx, C_FOLD + idx] = 1.0
    f[64 + idx, C_FOLD + idx] = 1.0
    f[idx, C_EXP + idx] = 1.0
    f[idx, C_EXP + 64 + idx] = 1.0
    f[:, C_I128:C_I128 + 128] = np.eye(128, dtype=np.float32)
    # embed stationary: row q -> out 0:64, row q+1 -> out 64:128 (q = 0, 32)
    for q in (0, 32):
        f[q, C_S2:C_S2 + 64] = 1.0
        f[q + 1, C_S2 + 64:C_S2 + 128] = 1.0
    ws = [inp['v_w1'], inp['v_w2'], inp['v_w3'], inp['v_w4']]
    for i in range(L):
        for k in range(4):
            c = C_VW + (i * 4 + k) * 64
            f[:64, c:c + 64] = ws[k][i]
            f[64:, c:c + 64] = ws[k][i]
    f[0:2, C_XW:C_XW + 64] = inp['v_lin0_w']
    f[64:66, C_XW:C_XW + 64] = inp['v_lin0_w']

    for i in range(L):
        bfc[:64, 128 * i:128 * i + 64] = inp['e_w'][i]
        bfc[64:, 128 * i + 64:128 * i + 128] = inp['e_w'][i]
    bfc[:64, 384] = inp['e_lin1_w'][:, 0]
    bfc[64:, 385] = inp['e_lin1_w'][:, 0]
    bfc[:, 386:514] = np.eye(128, dtype=np.float32)
    return f, bfc.astype(ml_dtypes.bfloat16)


def _diag_ap(t_ap, n_src, start=0):
    """AP over diag cols: start, start+201, ... (n_src entries), all 128 parts."""
    return bass.AP(t_ap.tensor, t_ap.offset + start,
                   [[t_ap.ap[0][0], 128], [N + 1, n_src]])


def build_nc():
    nc = bacc.Bacc(None, target_bir_lowering=False, debug=False,
                   num_devices=NCORES)
    x_d = nc.declare_dram_parameter("x", [BC, N, 2], F32, isOutput=False)
    adj_d = nc.declare_dram_parameter("adj", [BC, N, N], F32, isOutput=False)
    cf_d = nc.declare_dram_parameter("cf", [128, CF], F32, isOutput=False)
    cb_d = nc.declare_dram_parameter("cb", [128, CB], BF16, isOutput=False)
    out_d = nc.declare_dram_parameter("out", [BC, N, N], F32, isOutput=True)

    rg = [list(range(NCORES))]

    with tile.TileContext(nc) as tc:
        with (
            tc.tile_pool(name="big", bufs=1) as big,
            tc.tile_pool(name="sb", bufs=1) as sb,
            tc.tile_pool(name="scr", bufs=2) as scr,
            tc.tile_pool(name="ps_z", bufs=4, space="PSUM") as ps_z,
            tc.tile_pool(name="ps_s", bufs=2, space="PSUM") as ps_s,
            tc.tile_pool(name="ps_o", bufs=2, space="PSUM") as ps_o,
            tc.tile_pool(name="dram", bufs=1, space="DRAM") as dram,
        ):
            # ---------- persistent tiles ----------
            w_sb = big.tile([128, NCOL], BF16, tag="w")
            z_sb = big.tile([128, NCOL], BF16, tag="bigz")
            cf = sb.tile([128, CF], F32)
            cb = sb.tile([128, CB], BF16)
            nc.sync.dma_start(cf[:], cf_d[:])
            nc.sync.dma_start(cb[:], cb_d[:])

            h = sb.tile([128, N], F32)
            x1f = sb.tile([128, N], F32)
            x2b = sb.tile([128, N], BF16)
            x3b = sb.tile([128, N], BF16)
            x4b = sb.tile([128, N], BF16)
            x4d = sb.tile([128, 2 * N], BF16)
            pooled = sb.tile([128, N], F32)
            zv = sb.tile([128, N], F32)
            dtile = sb.tile([128, N], F32)
            vjunk = sb.tile([128, N], F32)
            scol = sb.tile([128, NZC], F32)
            sscol = sb.tile([128, NZC // SSG], F32)
            stats_e = sb.tile([128, 6], F32)
            tmp2 = sb.tile([64, 2], F32)
            msq = sb.tile([64, 2], F32)
            var1 = sb.tile([64, 1], F32)
            sd1 = sb.tile([64, 1], F32)
            inv1 = sb.tile([64, 1], F32)
            prm = sb.tile([64, 2], F32)
            scr_e = (msq, var1, sd1, inv1, prm, "psm")
            msqv = sb.tile([64, 2], F32)
            var1v = sb.tile([64, 1], F32)
            sd1v = sb.tile([64, 1], F32)
            inv1v = sb.tile([64, 1], F32)
            prmv = sb.tile([64, 2], F32)
            scr_v = (msqv, var1v, sd1v, inv1v, prmv, "psm")
            pe_sb = sb.tile([128, 2], F32)
            pv_sb = sb.tile([128, 2], F32)
            ar_sb = sb.tile([64, 6], F32)
            zer = sb.tile([2, N], F32)
            nc.vector.memset(zer[:], 0.0)

            def ccol(c, p0=0, p1=128):
                return cf[p0:p1, c:c + 1]

            # ---------- init: h embed ----------
            xt = sb.tile([128, N], F32)
            nc.vector.memset(xt[:], 0.0)
            xr = x_d[:].rearrange("b n c -> b c n")
            nc.sync.dma_start(xt[0:2, :], xr[0])
            nc.sync.dma_start(xt[64:66, :], xr[1])
            ph = ps_s.tile([128, N], F32, tag="psm")
            nc.tensor.matmul(ph[0:64, :], cf[0:2, C_XW:C_XW + 64],
                             xt[0:2, :], start=True, stop=True)
            nc.tensor.matmul(ph[64:128, :], cf[64:66, C_XW:C_XW + 64],
                             xt[64:66, :], start=True, stop=True)
            nc.scalar.activation(h[:], ph[:], AF.Lrelu, bias=ccol(C_VB0),
                                 scale=1.0, alpha=0.01)

            # ---------- init: w embed ----------
            # adj layout [128, 20000]: rows {0,1}=g0/g1 first half,
            # rows {32,33}=g0/g1 second half (quadrant-aligned for the PE).
            # One f32r matmul per chunk broadcasts both graphs.
            adj_sb = big.tile([128, NCOL // 2], F32R, tag="bigz")
            s2t = sb.tile([34, 128], F32R)
            nc.sync.dma_start(s2t[:],
                              cf_d[0:34, C_S2:C_S2 + 128].bitcast(F32R))
            af = adj_d[:].rearrange("b u v -> b (u v)").bitcast(F32R)
            half = NCOL // 2
            qq = half // 4
            dma_engs = [nc.sync, nc.scalar, nc.gpsimd, nc.sync]
            for g in range(2):
                for hh in range(2):
                    p0 = 32 * hh + g
                    for pc in range(4):
                        dma_engs[pc].dma_start(
                            adj_sb[p0:p0 + 1, pc * qq:(pc + 1) * qq],
                            af[g:g + 1, hh * half + pc * qq:
                               hh * half + (pc + 1) * qq])
            for hh in range(2):
                p0 = 32 * hh
                for c in range(half // ECH):
                    pe = ps_z.tile([128, ECH], F32, tag="pz")
                    cs = slice(c * ECH, (c + 1) * ECH)
                    nc.tensor.matmul(pe[:], s2t[p0:p0 + 2, :],
                                     adj_sb[p0:p0 + 2, cs],
                                     start=True, stop=True,
                                     tile_position=(p0, 0))
                    ho = hh * half
                    wcols = w_sb[:, ho + c * ECH:ho + (c + 1) * ECH]
                    if c % 5 < 3:
                        nc.scalar.activation(wcols, pe[:], AF.Lrelu,
                                             bias=ccol(C_EB0),
                                             scale=ccol(C_EW0), alpha=0.01)
                    else:
                        # DVE 3-op lrelu: t = s*pe + b; w = max(t, 0.01*t)
                        et = scr.tile([128, ECH], BF16, tag="sg")
                        eu = scr.tile([128, ECH], BF16, tag="pr")
                        nc.vector.scalar_tensor_tensor(
                            et[:], pe[:], ccol(C_EW0),
                            ccol(C_EB0).broadcast_to([128, ECH]),
                            OP.mult, OP.add)
                        nc.vector.tensor_scalar_mul(eu[:], et[:], 0.01)
                        nc.vector.tensor_tensor(wcols, et[:], eu[:], OP.max)
            nc.vector.memset(_diag_ap(w_sb[:], N), 0.0)

            # ---------- helpers ----------
            def bn_params(ar_ap, gcol, bcol, inv_cnt, out_sb, scratch):
                """ar_ap [64,2]=(S,SS) global -> out_sb [128,2]=(g', b')."""
                msq, var1, sd1, inv1, prm, ptag = scratch
                nc.vector.tensor_scalar_mul(msq[:], ar_ap, inv_cnt)
                nc.vector.tensor_tensor(var1[:], msq[:, 0:1], msq[:, 0:1], OP.mult)
                nc.vector.tensor_tensor(var1[:], msq[:, 1:2], var1[:], OP.subtract)
                nc.vector.tensor_scalar_add(var1[:], var1[:], EPS)
                nc.scalar.sqrt(sd1[:], var1[:])
                nc.vector.reciprocal(inv1[:], sd1[:])
                nc.vector.tensor_tensor(prm[:, 0:1], ccol(gcol, 0, 64), inv1[:],
                                        OP.mult)
                nc.vector.tensor_tensor(prm[:, 1:2], msq[:, 0:1], prm[:, 0:1],
                                        OP.mult)
                nc.vector.tensor_tensor(prm[:, 1:2], ccol(bcol, 0, 64),
                                        prm[:, 1:2], OP.subtract)
                pp = ps_s.tile([128, 2], F32, tag=ptag)
                nc.tensor.matmul(pp[:], cf[0:64, C_EXP:C_EXP + 128], prm[:],
                                 start=True, stop=True)
                nc.scalar.copy(out_sb[:], pp[:])

            # final-output machinery: one apply chunk (ACH cols) = FPG final
            # mm chunks of OCH cols; mm col-base rotates over quadrants
            # {0,32,64,96} so 4 chunks pack one psum tile; one wide ACT adds
            # the bias for all 8 live rows at once -> 2 grouped DMAs.
            of = out_d[:].rearrange("b u v -> b (u v)")
            FPG = ACH // OCH  # final chunks per apply chunk = 4

            def final_out(ac):
                pog = ps_o.tile([128, OCH], F32, tag="pout")
                oc4 = scr.tile([128, OCH], F32, tag="oc4")
                for j in range(FPG):
                    c = ac * FPG + j
                    cols = slice(c * OCH, (c + 1) * OCH)
                    q = 32 * j
                    nc.tensor.matmul(pog[q:q + 2, :], cb[:, 384:386],
                                     w_sb[:, cols], start=True, stop=True,
                                     tile_position=(0, q))
                nc.vector.tensor_scalar_add(oc4[:], pog[:], ccol(C_EL1B))
                # rows {0,32,64,96}+g -> graph g, 4 consecutive OCH blocks
                for g in range(BC):
                    src = bass.AP(oc4.tensor, oc4[:].offset + g * oc4[:].ap[0][0],
                                  [[32 * oc4[:].ap[0][0], 4], [1, OCH]])
                    dst = bass.AP(of.tensor, of.offset + g * NCOL + ac * ACH,
                                  [[OCH, 4], [1, OCH]])
                    nc.sync.dma_start(dst, src)

            # ---------- layers ----------
            stage = int(os.environ.get("KSTAGE", "3"))
            nlay = {0: 0, 1: 1, 2: 1, 3: L}[stage]
            pool_on = stage >= 2
            for i in range(nlay):
                last = (i == L - 1) or not pool_on
                for k, (dst, bcol) in enumerate(((x1f, C_B1 + i), (x2b, C_B2 + i),
                                                 (x3b, C_B3 + i), (x4b, C_B4 + i))):
                    if last and k < 2:
                        continue
                    wc = C_VW + (i * 4 + k) * 64
                    px = ps_s.tile([128, N], F32, tag="psm")
                    nc.tensor.matmul(px[0:64, :], cf[0:64, wc:wc + 64],
                                     h[0:64, :], start=True, stop=True)
                    nc.tensor.matmul(px[64:128, :], cf[64:128, wc:wc + 64],
                                     h[64:128, :], start=True, stop=True,
                                     tile_position=(64, 64))
                    nc.scalar.activation(dst[:], px[:], AF.Identity,
                                         bias=ccol(bcol), scale=1.0)
                nc.vector.tensor_copy(x4d[:, 0:N], x4b[:])
                nc.vector.tensor_copy(x4d[:, N:2 * N], x4b[:])
                x4v = x4b[:].unsqueeze(1).broadcast_to([128, 2, N])

                # z pass interleaved with pooling chunks (keeps PE warm,
                # overlaps ACT sigmoid / DVE prod+max with PE z-matmuls)
                def pool_chunk(c):
                    cols = slice(c * PCH, (c + 1) * PCH)
                    u0 = c * PCH_S
                    sg = scr.tile([128, PCH], BF16, tag="sg")
                    nc.scalar.activation(sg[:], w_sb[:, cols], AF.Sigmoid)
                    pr = scr.tile([128, PCH], BF16, tag="pr")
                    r2 = x2b[:].unsqueeze(1).broadcast_to([128, PCH_S, N])
                    nc.vector.tensor_tensor(
                        pr[:].rearrange("p (u v) -> p u v", v=N),
                        sg[:].rearrange("p (u v) -> p u v", v=N), r2, OP.mult)
                    nc.vector.memset(_diag_ap(pr[:], PCH_S, start=u0), -1e30)
                    pr3 = pr[:].rearrange("p (u v) -> p u v", v=N)
                    ph2 = scr.tile([128, PCH_S * (N // 2)], BF16, tag="oc4")
                    ph23 = ph2[:].rearrange("p (u v) -> p u v", v=N // 2)
                    nc.vector.tensor_tensor(ph23, pr3[:, :, 0:N // 2],
                                            pr3[:, :, N // 2:N], OP.max)
                    nc.vector.reduce_max(
                        pooled[:, u0:u0 + PCH_S], ph23,
                        axis=mybir.AxisListType.X)

                zfrac = NZC // (NCOL // PCH)  # z chunks per pooling chunk
                for c in range(NZC):
                    cols = slice(c * ZCH, (c + 1) * ZCH)
                    u0 = c * 2
                    even = (c % 2 == 0)
                    pz = ps_z.tile([128, ZCH], F32, tag="pz")
                    nc.tensor.matmul(pz[:], cb[:, 128 * i:128 * i + 128],
                                     w_sb[:, cols], start=True, stop=False)
                    pz3 = pz[:].rearrange("p (u v) -> p u v", v=N)
                    r3 = x3b[:, u0:u0 + 2].unsqueeze(2).broadcast_to([128, 2, N])
                    nc.tensor.matmul(pz3, cb[:, 386:514], r3,
                                     start=False, stop=not even)
                    if even:
                        # x4 add via PE; ACT does copy + S-accum
                        nc.tensor.matmul(pz[:], cb[:, 386:514],
                                         x4d[:], start=False, stop=True)
                        nc.scalar.activation(z_sb[:, cols], pz[:], AF.Identity,
                                             bias=0.0, scale=1.0,
                                             accum_out=scol[:, c:c + 1])
                    else:
                        # x4 add fused into the DVE copy, with S-accum
                        nc.vector.scalar_tensor_tensor(
                            z_sb[:, cols].rearrange("p (u v) -> p u v", v=N),
                            pz3, 1.0, x4v, OP.mult, OP.add,
                            accum_out=scol[:, c:c + 1])
                    if c % SSG == SSG - 1:
                        g4 = c // SSG
                        zc = slice(g4 * SSG * ZCH, (c + 1) * ZCH)
                        junk = scr.tile([128, SSG * ZCH], BF16, tag="lr")
                        if g4 % 3 != 2:
                            nc.scalar.activation(junk[:], z_sb[:, zc],
                                                 AF.Square,
                                                 accum_out=sscol[:, g4:g4 + 1])
                        else:
                            nc.vector.scalar_tensor_tensor(
                                junk[:], z_sb[:, zc], 1.0, z_sb[:, zc],
                                OP.mult, OP.mult,
                                accum_out=sscol[:, g4:g4 + 1])
                    # interleave the head pool chunks; the tail runs after
                    # the e-stats AllReduce dispatch to hide its latency
                    if not last and c % zfrac == zfrac - 1:
                        p = c // zfrac
                        if p < NPOOL - PTAIL:
                            pool_chunk(p)
                # e-stats: dense S/SS then diag S_d/SS_d (diag cols of z are
                # exactly x3b+x4b; correction subtracted after AllReduce)
                nc.vector.reduce_sum(stats_e[:, 0:1], scol[:],
                                     axis=mybir.AxisListType.X)
                nc.vector.reduce_sum(stats_e[:, 1:2], sscol[:],
                                     axis=mybir.AxisListType.X)
                nc.vector.scalar_tensor_tensor(dtile[:], x3b[:], 1.0, x4b[:],
                                               OP.mult, OP.add,
                                               accum_out=stats_e[:, 2:3])
                nc.vector.scalar_tensor_tensor(vjunk[:], dtile[:], 1.0, dtile[:],
                                               OP.mult, OP.mult,
                                               accum_out=stats_e[:, 3:4])
                pf_ps = ps_s.tile([64, 6], F32, tag="psm")
                nc.tensor.matmul(pf_ps[:, 0:4], cf[:, C_FOLD:C_FOLD + 64],
                                 stats_e[:, 0:4], start=True, stop=True)
                nc.scalar.copy(ar_sb[:, 0:4], pf_ps[:, 0:4])
                are_in = dram.tile([64, 4], F32, tag=f"arei{i}")
                are_out = dram.tile([64, 4], F32, tag=f"areo{i}")
                nc.sync.dma_start(are_in[:], ar_sb[:, 0:4])
                nc.gpsimd.collective_compute("AllReduce", OP.add,
                                             replica_groups=rg,
                                             ins=[are_in.opt()],
                                             outs=[are_out.opt()])
                ar_e = sb.tile([64, 4], F32, tag="areb")
                nc.gpsimd.dma_start(ar_e[:], are_out[:])

                if not last:
                    # pooling tail + v-stats + v AllReduce, all overlapping
                    # the e AllReduce latency
                    for p in range(NPOOL - PTAIL, NPOOL):
                        pool_chunk(p)
                    nc.vector.scalar_tensor_tensor(zv[:], x1f[:], 1.0, pooled[:],
                                                   OP.mult, OP.add,
                                                   accum_out=stats_e[:, 4:5])
                    nc.vector.scalar_tensor_tensor(vjunk[:], zv[:], 1.0, zv[:],
                                                   OP.mult, OP.mult,
                                                   accum_out=stats_e[:, 5:6])
                    pv_ps = ps_s.tile([64, 6], F32, tag="psm")
                    nc.tensor.matmul(pv_ps[:, 0:2], cf[:, C_FOLD:C_FOLD + 64],
                                     stats_e[:, 4:6], start=True, stop=True)
                    nc.scalar.copy(ar_sb[:, 4:6], pv_ps[:, 0:2])
                    arv_in = dram.tile([64, 2], F32, tag=f"arvi{i}")
                    arv_out = dram.tile([64, 2], F32, tag=f"arvo{i}")
                    nc.sync.dma_start(arv_in[:], ar_sb[:, 4:6])
                    nc.gpsimd.collective_compute("AllReduce", OP.add,
                                                 replica_groups=rg,
                                                 ins=[arv_in.opt()],
                                                 outs=[arv_out.opt()])
                    ar_v = sb.tile([64, 2], F32, tag="arvb")
                    nc.gpsimd.dma_start(ar_v[:], arv_out[:])

                # e-BN params; apply to w
                nc.vector.tensor_tensor(tmp2[:], ar_e[:, 0:2], ar_e[:, 2:4],
                                        OP.subtract)
                bn_params(tmp2[:], C_EG + i, C_EB + i, 1.0 / CNT_E, pe_sb, scr_e)
                lr0 = None
                nach = NCOL // ACH
                kdve = 1 if i == L - 1 else 4
                for c in range(nach):
                    cols = slice(c * ACH, (c + 1) * ACH)
                    lr = scr.tile([128, ACH], BF16, tag="lr")
                    if c >= nach - kdve:
                        ts1 = scr.tile([128, ACH], BF16, tag="sg")
                        ts2 = scr.tile([128, ACH], BF16, tag="pr")
                        nc.vector.tensor_scalar(ts1[:], z_sb[:, cols],
                                                pe_sb[:, 0:1], pe_sb[:, 1:2],
                                                OP.mult, OP.add)
                        nc.vector.tensor_scalar_mul(ts2[:], ts1[:], 0.01)
                        nc.vector.tensor_tensor(lr[:], ts1[:], ts2[:], OP.max)
                    else:
                        nc.scalar.activation(lr[:], z_sb[:, cols], AF.Lrelu,
                                             bias=pe_sb[:, 1:2],
                                             scale=pe_sb[:, 0:1], alpha=0.01)
                    if lr0 is None:
                        lr0 = lr
                    nc.vector.tensor_tensor(w_sb[:, cols], w_sb[:, cols],
                                            lr[:], OP.add)
                    if i == L - 1:
                        final_out(c)
                if not last:
                    nc.vector.memset(_diag_ap(w_sb[:], N), 0.0)
                    # WAW hook: pins the v-params chain behind the first apply
                    # chunk so its ACT ops can't be scheduled ahead of the
                    # apply (the sim underestimates AllReduce latency)
                    nc.vector.tensor_copy(msqv[0:1, 0:1], lr0[0:1, 0:1])
                    bn_params(ar_v[:], C_VG + i, C_VB + i, 1.0 / CNT_V, pv_sb, scr_v)
                    hup = sb.tile([128, N], F32, tag="hup")
                    nc.scalar.activation(hup[:], zv[:], AF.Lrelu,
                                         bias=pv_sb[:, 1:2], scale=pv_sb[:, 0:1],
                                         alpha=0.01)
                    nc.vector.tensor_tensor(h[:], h[:], hup[:], OP.add)

            # final-output epilogue: covered by final_out() calls inside
            # the last apply loop; here only the diagonal zeroing remains.
            dd = bass.AP(of.tensor, of.offset, [[NCOL, BC], [N + 1, N]])
            nc.sync.dma_start(dd, zer[:])
    nc.compile()
    return nc


def _get_nc():
    if "nc" not in _CACHE:
        _CACHE["nc"] = build_nc()
    return _CACHE["nc"]


def run(inputs, **kw):
    inp = {k: np.asarray(v, np.float32) for k, v in inputs.items()}
    cfh, cbh = _build_consts(inp)
    nc = _get_nc()
    in_maps = []
    for c in range(NCORES):
        sl = slice(c * BC, (c + 1) * BC)
        in_maps.append({
            "x": np.ascontiguousarray(inp['x'][sl]),
            "adj": np.ascontiguousarray(inp['adj'][sl]),
            "cf": cfh, "cb": cbh,
        })
    res = run_bass_kernel_spmd(nc, in_maps, core_ids=list(range(NCORES)), **kw)
    out = np.concatenate([res.results[c]["out"] for c in range(NCORES)], axis=0)
    return out, res


def kernel(**inputs) -> np.ndarray:
    out, _ = run(inputs)
    return out
